# revision 8
# baseline (speedup 1.0000x reference)
"""CRF loss kernel for Trainium2 (8 NeuronCores, data-parallel over batch).

Math (faithful to the reference):
  loss = (forscore - tg_energy) / B
  tg_energy = B*trans[0,START] + sum_bt scores[b,t,0] + sum_bt trans[0, gold[b,t]]
    (the reference's torch.gather-on-flattened-(L*L) quirk reduces to row 0)
  forscore = sum_b fs_T[b, END], where fs is the standard CRF forward recurrence
    fs_{t+1}[j] = logsumexp_i(fs_t[i] + scores[t,i] + trans[i,j]), fs_0 = trans[START,:]

Device algorithm, linear space with E = exp(trans) (bf16 matmuls, f32 PSUM).
The forward chain (t ascending) and the adjoint chain (t descending) are
PARTITION-STACKED into one [96, 8] state so each of the 256 rounds is exactly
one DVE multiply + one PE matmul against a constant block-diagonal stationary:
  partitions 0-47  (A): w_{t+1} = E^T (w_t * sA_t),  w_0 = exp(trans[START,:])
  partitions 48-95 (B): r_t     = sB_t * (E r_{t+1}), seeded r via exp(trans[:,END])
  lhsT = blockdiag(E, E^T)  (exp of [[trans, -100],[-100, trans^T]])
The final round uses the SWAPPED stationary [[0, E],[E^T, 0]] so that the
product (E r_256) lands on partitions 0-47, aligned with yA_255 for the
junction dot product  e_END^T w_T = (E r_256)^T yA_255.

Magnitude control: at each chunk boundary (CHS schedule) one [96,2]-stationary
matmul measures zA/zB = per-chain column sums of y; 1/z (bf16) is broadcast
back across each 48-partition block by a K=2 matmul and applied as a one-step
multiplicative jolt to the exp'd-score stream two chunks later. The f32 z
values stream to DRAM and the host adds back sum(log z):
  fs_T[b] = log(dot[b]) + T*delta + sum_c log zA[c,b] + sum_c log zB[c,b].

gold-energy pieces (tag histogram of gold, scores[:,:,0] sum) are folded into
the DVE/PE slack of the main loop.

Per-core layout: tags on partitions (2 x 48), local batch (8) on the free dim.
mask is all ones per the problem spec (fill: ones), so the mask gating is the
identity and is not materialized on device.
"""

import numpy as np

B, T, L = 64, 512, 48
START, PAD, END = 46, 45, 47
NCORES = 8
BL = B // NCORES          # 8 batch elements per core
H = T // 2                # rounds (each round advances both chains one step)
P2 = 2 * L                # 96 partitions: A block + B block
CHS = [8, 24] + [32] * 7  # renorm chunk sizes (small first chunk -> the first
                          # DMA+exp gating the chain start is tiny)
SOFF = [sum(CHS[:i]) for i in range(len(CHS))]
NCH2 = len(CHS)           # chunks
LAG = 2                   # feedback delay (chunks) for the 1/z correction
NZ = NCH2 - LAG           # chunks whose z is measured/applied/logged
DELTA = 5.0               # static per-step log shift folded into exp(scores)

_NC_CACHE = {}


def build_nc():
    import concourse.bacc as bacc
    import concourse.mybir as mybir
    import concourse.tile as tile

    f32 = mybir.dt.float32
    bf16 = mybir.dt.bfloat16
    AF = mybir.ActivationFunctionType
    AL = mybir.AluOpType
    AX = mybir.AxisListType

    nc = bacc.Bacc("TRN2", target_bir_lowering=False, debug=False)

    s_dram = nc.dram_tensor("s_tr", [P2, H * BL], f32, kind="ExternalInput")
    m_d = nc.dram_tensor("mraw", [P2, P2], f32, kind="ExternalInput")
    msw_d = nc.dram_tensor("mswraw", [P2, P2], f32, kind="ExternalInput")
    v0_d = nc.dram_tensor("v0raw", [P2, 1], f32, kind="ExternalInput")
    t0_d = nc.dram_tensor("t0col", [L, 1], f32, kind="ExternalInput")
    ones2_d = nc.dram_tensor("ones2raw", [P2, 2], f32, kind="ExternalInput")
    mask2_d = nc.dram_tensor("mask2raw", [2, P2], f32, kind="ExternalInput")
    goldf_d = nc.dram_tensor("goldf", [128, 32], f32, kind="ExternalInput")
    sc0_d = nc.dram_tensor("sc0", [128, 32], f32, kind="ExternalInput")
    iota_d = nc.dram_tensor("iotaf", [128, L], f32, kind="ExternalInput")

    # out rows (partitions 0/1):
    #   row0: [ zA (NZ*BL) | dot (BL) | tgg | sc0_sum ]
    #   row1: [ zB (NZ*BL) | unused ]
    OW = NZ * BL + BL + 2
    ZD = NZ * BL              # dot offset
    ZS = ZD + BL              # (tgg, sc0_sum) offset
    out_d = nc.dram_tensor("out_all", [2, OW], f32, kind="ExternalOutput")

    with tile.TileContext(nc) as tc:
        with (
            tc.tile_pool(name="const", bufs=1) as cpool,
            tc.tile_pool(name="sraw", bufs=4) as rpool,
            tc.tile_pool(name="sexp", bufs=4) as epool,
            tc.tile_pool(name="yy", bufs=4) as ypool,
            tc.tile_pool(name="small", bufs=4) as smpool,
            tc.tile_pool(name="oh", bufs=2) as ohpool,
            tc.tile_pool(name="wps", bufs=2, space="PSUM") as wpool,
            tc.tile_pool(name="zps", bufs=2, space="PSUM") as zpool,
            tc.tile_pool(name="cntps", bufs=1, space="PSUM") as cntpool,
            tc.tile_pool(name="cbps", bufs=2, space="PSUM") as cbpool,
        ):
            # ---- startup-critical DMAs first (raw chunk 0 + v0 + blockdiag
            # gate round 0); other constants ride the gpsimd DMA queue ----
            raw0 = rpool.tile([P2, CHS[0] * BL], f32, tag="raw")
            nc.sync.dma_start(raw0[:], s_dram[:, 0:CHS[0] * BL])
            v0r = cpool.tile([P2, 1], f32)
            nc.sync.dma_start(v0r[:], v0_d[:])
            mraw_sb = cpool.tile([P2, P2], f32)
            nc.sync.dma_start(mraw_sb[:], m_d[:])

            msw_sb = cpool.tile([P2, P2], f32)
            nc.gpsimd.dma_start(msw_sb[:], msw_d[:])
            t0c = cpool.tile([L, 1], f32)
            nc.gpsimd.dma_start(t0c[:], t0_d[:])

            zero96 = cpool.tile([P2, 1], f32)
            nc.vector.memset(zero96[:], 0.0)
            negd96 = cpool.tile([P2, 1], f32)
            nc.vector.memset(negd96[:], -DELTA)

            # prefetch the Exp activation table while the DMAs run
            warm_act = cpool.tile([P2, 1], f32)
            nc.scalar.activation(warm_act[:], zero96[:], AF.Exp, bias=zero96[:])

            # chain inits and stationaries (Exp of the raw DMAs)
            v0 = cpool.tile([P2, 1], f32)
            nc.scalar.activation(v0[:], v0r[:], AF.Exp, bias=zero96[:])
            m_bf = cpool.tile([P2, P2], bf16)
            nc.scalar.activation(m_bf[:], mraw_sb[:], AF.Exp, bias=zero96[:])
            msw_bf = cpool.tile([P2, P2], bf16)
            nc.scalar.activation(msw_bf[:], msw_sb[:], AF.Exp, bias=zero96[:])

            # z-measure stationary: col0 sums partitions 0-47, col1 sums 48-95
            # (partition-sliced memsets need 32-aligned bases, so these come
            # from DRAM instead)
            ones2f = cpool.tile([P2, 2], f32)
            nc.gpsimd.dma_start(ones2f[:], ones2_d[:])
            ones2 = cpool.tile([P2, 2], bf16)
            nc.vector.tensor_copy(ones2[:], ones2f[:])
            # 1/z broadcast stationary: row0 -> partitions 0-47, row1 -> 48-95
            mask2 = cpool.tile([2, P2], f32)
            nc.gpsimd.dma_start(mask2[:], mask2_d[:])

            ones48f = cpool.tile([L, 1], f32)
            nc.vector.memset(ones48f[:], 1.0)
            ones128b = cpool.tile([128, 1], bf16)
            nc.gpsimd.memset(ones128b[:], 1.0)
            ones128f = cpool.tile([128, 1], f32)
            nc.gpsimd.memset(ones128f[:], 1.0)
            zbuf = cpool.tile([2, OW], f32)
            nc.vector.memset(zbuf[:], 0.0)

            # gold-histogram inputs (consumed from chunk 1 onward, off-path)
            iota_sb = cpool.tile([128, L], f32)
            nc.gpsimd.dma_start(iota_sb[:], iota_d[:])
            goldf_sb = cpool.tile([128, 32], f32)
            nc.gpsimd.dma_start(goldf_sb[:], goldf_d[:])
            sc0_sb = cpool.tile([128, 32], f32)
            nc.gpsimd.dma_start(sc0_sb[:], sc0_d[:])
            cnt_ps = cntpool.tile([L, 1], f32)

            # ---- merged 256-round chain ----
            cbmap = {}
            w_prev = None
            y = None
            pending_fb = []   # feedback ops deferred into the next chunk so
                              # the in-order PE queue isn't head-blocked on DVE
            pending_z = []    # z-measure matmuls, deferred one round likewise
            hist_cc = 0
            for c in range(NCH2):
                K = CHS[c]
                s0, s1 = SOFF[c], SOFF[c] + K
                if c == 0:
                    raw = raw0
                else:
                    raw = rpool.tile([P2, K * BL], f32, tag="raw")
                    nc.sync.dma_start(raw[:], s_dram[:, s0 * BL:s1 * BL])
                se = epool.tile([P2, K, BL], f32, tag="se")
                nc.scalar.activation(
                    se[:].rearrange("p a b -> p (a b)"), raw[:], AF.Exp,
                    bias=negd96[:])

                if c in cbmap:
                    s0c = smpool.tile([P2, BL], f32, tag="s0c")
                    nc.vector.tensor_tensor(
                        s0c[:], se[:, 0, :], cbmap.pop(c)[:], AL.mult)
                else:
                    s0c = None

                for k in range(K):
                    if k == 1 and pending_z:
                        for zb in pending_z:
                            zb()
                        pending_z = []
                    if k == 5 and pending_fb:
                        for fb in pending_fb:
                            fb()
                        pending_fb = []

                    # the one DVE op of the round
                    src = s0c[:] if (k == 0 and s0c is not None) else se[:, k, :]
                    y = ypool.tile([P2, BL], bf16, tag="y")
                    if c == 0 and k == 0:
                        nc.vector.tensor_scalar_mul(y[:], se[:, 0, :], v0[:])
                    else:
                        nc.vector.tensor_tensor(y[:], w_prev[:], src, AL.mult)

                    # gold-histogram pieces folded into the loop's DVE/PE slack
                    if c >= 1 and k % 4 == 2 and hist_cc < 32:
                        cc = hist_cc
                        hist_cc += 1
                        oh = ohpool.tile([128, L], bf16, tag="oh")
                        nc.vector.tensor_scalar(
                            oh[:], iota_sb[:], goldf_sb[:, cc:cc + 1], None,
                            AL.is_equal)
                        nc.tensor.matmul(
                            cnt_ps[:], oh[:], ones128b[:],
                            start=(cc == 0), stop=(cc == 31))
                        if cc == 31:
                            # tg epilogue, inside the loop so it overlaps the
                            # remaining chunks instead of serializing after
                            cnt_sb = smpool.tile([L, 1], f32, tag="cnt")
                            nc.vector.tensor_copy(cnt_sb[:], cnt_ps[:])
                            tgg_ps = zpool.tile([1, 1], f32, tag="z")
                            nc.tensor.matmul(
                                tgg_ps[:], cnt_sb[:], t0c[:],
                                start=True, stop=True)
                            nc.vector.tensor_copy(
                                zbuf[0:1, ZS:ZS + 1], tgg_ps[:])
                            red = smpool.tile([128, 1], f32, tag="red")
                            nc.vector.reduce_sum(red[:], sc0_sb[:], axis=AX.X)
                            sc_ps = zpool.tile([1, 1], f32, tag="z")
                            nc.tensor.matmul(
                                sc_ps[:], red[:], ones128f[:],
                                start=True, stop=True)
                            nc.vector.tensor_copy(
                                zbuf[0:1, ZS + 1:ZS + 2], sc_ps[:])

                    # the one PE op of the round (swapped stationary at the
                    # very end so (E r_256) lands partition-aligned with yA)
                    last = (c == NCH2 - 1 and k == K - 1)
                    w_prev = wpool.tile([P2, BL], f32, tag="w")
                    nc.tensor.matmul(
                        w_prev[:], (msw_bf if last else m_bf)[:], y[:],
                        start=True, stop=True)

                # chunk-end magnitude measurement + delayed 1/z feedback.
                # The z matmul + copy run one round into the next chunk and
                # the reciprocal + broadcast five rounds later, so neither
                # the PE nor the DVE queue head-blocks at the chunk seam.
                if c < NZ:
                    def _zb(ytile=y, c=c):
                        z_ps = zpool.tile([2, BL], f32, tag="z")
                        nc.tensor.matmul(
                            z_ps[:], ones2[:], ytile[:], start=True, stop=True)
                        nc.vector.tensor_copy(
                            zbuf[:, c * BL:(c + 1) * BL], z_ps[:])

                        def _fb(z_ps=z_ps, c=c):
                            zr = smpool.tile([2, BL], f32, tag="zr")
                            nc.vector.reciprocal(zr[:], z_ps[:])
                            cbt = cbpool.tile([P2, BL], f32, tag="cb")
                            nc.tensor.matmul(
                                cbt[:], mask2[:], zr[:], start=True, stop=True)
                            cbmap[c + LAG] = cbt
                        pending_fb.append(_fb)
                    pending_z.append(_zb)

            # junction dot product: e_END^T w_T = (E r_256)^T yA_255
            dprod = smpool.tile([L, BL], f32, tag="dprod")
            nc.vector.tensor_tensor(dprod[:], w_prev[0:L, :], y[0:L, :], AL.mult)
            d_ps = zpool.tile([1, BL], f32, tag="z")
            nc.tensor.matmul(d_ps[:], ones48f[:], dprod[:], start=True, stop=True)
            nc.vector.tensor_copy(zbuf[0:1, ZD:ZD + BL], d_ps[:])
            nc.sync.dma_start(out_d[:], zbuf[:])

    nc.compile()
    return nc


def _get_nc():
    if "nc" not in _NC_CACHE:
        _NC_CACHE["nc"] = build_nc()
    return _NC_CACHE["nc"]


def make_in_maps(scores, gold_target, transitions):
    scores = np.asarray(scores, dtype=np.float32)
    gold = np.asarray(gold_target)
    trans = np.ascontiguousarray(np.asarray(transitions, dtype=np.float32))

    mraw = np.full((P2, P2), -100.0, dtype=np.float32)
    mraw[0:L, 0:L] = trans
    mraw[L:P2, L:P2] = trans.T
    mswraw = np.full((P2, P2), -100.0, dtype=np.float32)
    mswraw[0:L, L:P2] = trans
    mswraw[L:P2, 0:L] = trans.T
    v0raw = np.empty((P2, 1), dtype=np.float32)
    v0raw[0:L, 0] = trans[START, :]
    v0raw[L:P2, 0] = trans[:, END]
    t0col = np.ascontiguousarray(trans[0, :, None])
    ones2raw = np.zeros((P2, 2), dtype=np.float32)
    ones2raw[0:L, 0] = 1.0
    ones2raw[L:P2, 1] = 1.0
    mask2raw = np.zeros((2, P2), dtype=np.float32)
    mask2raw[0, 0:L] = 1.0
    mask2raw[1, L:P2] = 1.0
    iota = np.ascontiguousarray(
        np.broadcast_to(np.arange(L, dtype=np.float32)[None, :], (128, L)))

    in_maps = []
    for c in range(NCORES):
        sc = scores[c * BL:(c + 1) * BL]                     # (BL, T, L)
        sA = np.ascontiguousarray(sc.transpose(2, 1, 0))     # (L, T, BL)
        s_tr = np.empty((P2, H * BL), dtype=np.float32)
        s_tr[0:L] = sA[:, :H, :].reshape(L, H * BL)
        s_tr[L:P2] = sA[:, ::-1, :][:, :H, :].reshape(L, H * BL)
        goldf = np.ascontiguousarray(
            gold[c * BL:(c + 1) * BL].astype(np.float32).reshape(128, 32))
        sc0 = np.ascontiguousarray(sc[:, :, 0].astype(np.float32).reshape(128, 32))
        in_maps.append({
            "s_tr": s_tr, "mraw": mraw, "mswraw": mswraw, "v0raw": v0raw,
            "t0col": t0col, "ones2raw": ones2raw, "mask2raw": mask2raw,
            "goldf": goldf, "sc0": sc0, "iotaf": iota,
        })
    return in_maps


def combine_outputs(results, transitions):
    trans = np.asarray(transitions, dtype=np.float64)
    forscore = 0.0
    tg_energy = 0.0
    nz = NZ * BL
    for c in range(NCORES):
        out = np.asarray(results[c]["out_all"], dtype=np.float64)
        zv = out[:, :nz].reshape(2, NZ, BL)
        dv = out[0, nz:nz + BL]
        tgg, sc0s = out[0, nz + BL], out[0, nz + BL + 1]
        fs_end = (np.log(dv) + DELTA * T
                  + np.log(zv[0]).sum(axis=0)
                  + np.log(zv[1]).sum(axis=0))
        forscore += fs_end.sum()
        tg_energy += tgg + sc0s + BL * trans[0, START]
    return np.float32((forscore - tg_energy) / B)


def kernel(scores, gold_target, mask, transitions):
    from concourse.bass_utils import run_bass_kernel_spmd

    nc = _get_nc()
    in_maps = make_in_maps(scores, gold_target, transitions)
    res = run_bass_kernel_spmd(nc, in_maps, list(range(NCORES)))
    return combine_outputs(res.results, transitions)


# revision 15
# speedup vs baseline: 1.0320x; 1.0320x over previous
"""CRF loss kernel for Trainium2 (8 NeuronCores, data-parallel over batch).

Math (faithful to the reference):
  loss = (forscore - tg_energy) / B
  tg_energy = B*trans[0,START] + sum_bt scores[b,t,0] + sum_bt trans[0, gold[b,t]]
    (the reference's torch.gather-on-flattened-(L*L) quirk reduces to row 0)
  forscore = sum_b fs_T[b, END], where fs is the standard CRF forward recurrence
    fs_{t+1}[j] = logsumexp_i(fs_t[i] + scores[t,i] + trans[i,j]), fs_0 = trans[START,:]

Device algorithm, linear space with E = exp(trans) (f32 PSUM accumulators):
  forward half  (t = 0..T/2-1):    w_{t+1} = E^T (w_t  * s_t),  w_0 = exp(trans[START,:])
  backward half (t = T-1..T/2):    r_t     = s_t * (E r_{t+1}), r_T = e_END  (adjoint)
  e_END^T w_T = r_{T/2}^T w_{T/2}  -> one dot product at the junction.
The two 256-step chains are independent, so each one's (DVE mul -> PE matmul ->
sem) latency hides inside the other's gaps: ~256 dependent rounds, not 512.
The backward half's score stream is reversed on the HOST so both chains read
their exp'd scores with dense unit-stride (a reversed-stride DVE read costs
~50% extra on cayman).
s_t = exp(scores_t - delta) is produced by the scalar engine (Exp only — no
activation-table thrashing).

Magnitude control: at each chunk boundary (CHS schedule) each chain measures
z = 1^T y via a tiny matmul; 1/z (vector reciprocal, bf16) is broadcast across
partitions by a K=1 matmul and applied as a one-step multiplicative jolt to
that chain's exp'd-score stream two chunks later. The f32 z values stream to
DRAM and the host adds back sum(log z) over the applied corrections:
  fs_T[b] = log(dot[b]) + T*delta + sum_c log zA[c,b] + sum_c log zB[c,b].

gold-energy pieces: the tag histogram runs as fused (iota==gold)+acc
scalar_tensor_tensor ops in DVE slack (no per-piece PE matmuls), reduced by
one matmul near the end; the host does the final 48-dot with trans[0,:].

Per-core layout: tags on partitions (48), local batch (8) on the free dim.
mask is all ones per the problem spec (fill: ones), so the mask gating
(where(mask, nxt, fs)) is the identity and is not materialized on device.
"""

import numpy as np

B, T, L = 64, 512, 48
START, PAD, END = 46, 45, 47
NCORES = 8
BL = B // NCORES          # 8 batch elements per core
H = T // 2                # steps per chain
CHS = [8, 24] + [32] * 7  # renorm chunk sizes (small first chunk -> the first
                          # DMA+exp gating the chain start is tiny)
SOFF = [sum(CHS[:i]) for i in range(len(CHS))]
NCH2 = len(CHS)           # chunks per chain
LAG = 2                   # feedback delay (chunks) for the 1/z correction
NZ = NCH2 - LAG           # chunks whose z is measured/applied/logged
DELTA = 5.0               # static per-step log shift folded into exp(scores)
FP8 = False               # stationary dtype for the chain matmuls

_NC_CACHE = {}


def build_nc():
    import concourse.bacc as bacc
    import concourse.mybir as mybir
    import concourse.tile as tile

    f32 = mybir.dt.float32
    bf16 = mybir.dt.bfloat16
    stat_dt = mybir.dt.float8e4 if FP8 else bf16
    AF = mybir.ActivationFunctionType
    AL = mybir.AluOpType
    AX = mybir.AxisListType

    nc = bacc.Bacc("TRN2", target_bir_lowering=False, debug=False)

    s_dram = nc.dram_tensor("s_tr", [2 * L, H * BL], f32, kind="ExternalInput")
    trans_d = nc.dram_tensor("trans", [L, L], f32, kind="ExternalInput")
    transT_d = nc.dram_tensor("transT", [L, L], f32, kind="ExternalInput")
    goldf_d = nc.dram_tensor("goldf", [128, 32], f32, kind="ExternalInput")
    sc0_d = nc.dram_tensor("sc0", [128, 32], f32, kind="ExternalInput")
    iota_d = nc.dram_tensor("iotaf", [128, L], f32, kind="ExternalInput")

    # out row: [ zA|zB (16/chunk) * NZ | dot(8) | sc0_sum | cnt(48) ]
    ZD = NZ * 2 * BL          # dot offset
    ZS = ZD + BL              # sc0_sum offset
    HC = ZS + 1               # histogram row offset
    OW = HC + L
    out_d = nc.dram_tensor("out_all", [1, OW], f32, kind="ExternalOutput")

    with tile.TileContext(nc) as tc:
        with (
            tc.tile_pool(name="const", bufs=1) as cpool,
            tc.tile_pool(name="sraw", bufs=4) as rpool,
            tc.tile_pool(name="sexp", bufs=4) as epool,
            tc.tile_pool(name="yy", bufs=4) as ypool,
            tc.tile_pool(name="small", bufs=4) as smpool,
            tc.tile_pool(name="acc", bufs=2) as accpool,
            tc.tile_pool(name="wps", bufs=2, space="PSUM") as wpool,
            tc.tile_pool(name="qps", bufs=2, space="PSUM") as qpool,
            tc.tile_pool(name="zps", bufs=2, space="PSUM") as zpool,
            tc.tile_pool(name="cntps", bufs=1, space="PSUM") as cntpool,
            tc.tile_pool(name="cbps", bufs=1, space="PSUM") as cbpool,
        ):
            # ---- startup-critical DMAs first (transT gates the chain-A init
            # scalar; raw chunks gate the first steps); other constants ride
            # the (otherwise idle) gpsimd DMA queue ----
            raw0A = rpool.tile([L, CHS[0] * BL], f32, tag="rawA")
            nc.sync.dma_start(raw0A[:], s_dram[0:L, 0:CHS[0] * BL])
            transT_sb = cpool.tile([L, L], f32)
            nc.sync.dma_start(transT_sb[:], transT_d[:])

            trans_sb = cpool.tile([L, L], f32)
            nc.gpsimd.dma_start(trans_sb[:], trans_d[:])
            raw0B = rpool.tile([L, CHS[0] * BL], f32, tag="rawB")
            nc.gpsimd.dma_start(raw0B[:], s_dram[L:2 * L, 0:CHS[0] * BL])

            zero48 = cpool.tile([L, 1], f32)
            nc.vector.memset(zero48[:], 0.0)
            negd48 = cpool.tile([L, 1], f32)
            nc.vector.memset(negd48[:], -DELTA)

            # prefetch the Exp activation table while the DMAs run
            warm_act = cpool.tile([L, 1], f32)
            nc.scalar.activation(warm_act[:], zero48[:], AF.Exp, bias=zero48[:])

            # only two f32 columns are needed for the chain inits:
            # exp(trans[START,:]) = exp(transT[:,START]) and exp(trans[:,END])
            ETcol = cpool.tile([L, 1], f32)
            nc.scalar.activation(
                ETcol[:], transT_sb[:, START:START + 1], AF.Exp, bias=zero48[:])
            E_st = cpool.tile([L, L], stat_dt)
            nc.scalar.activation(E_st[:], trans_sb[:], AF.Exp, bias=zero48[:])
            Ecol = cpool.tile([L, 1], f32)
            nc.scalar.activation(
                Ecol[:], trans_sb[:, END:END + 1], AF.Exp, bias=zero48[:])
            ET_st = cpool.tile([L, L], stat_dt)
            nc.scalar.activation(ET_st[:], transT_sb[:], AF.Exp, bias=zero48[:])

            ones48b = cpool.tile([L, 1], bf16)
            nc.vector.memset(ones48b[:], 1.0)
            ones48f = cpool.tile([L, 1], f32)
            nc.vector.memset(ones48f[:], 1.0)
            ones1x48 = cpool.tile([1, L], bf16)
            nc.vector.memset(ones1x48[:], 1.0)
            ones128f = cpool.tile([128, 1], f32)
            nc.gpsimd.memset(ones128f[:], 1.0)
            zbuf = cpool.tile([1, OW], f32)
            nc.vector.memset(zbuf[:], 0.0)
            acc0 = cpool.tile([128, L], f32)
            nc.gpsimd.memset(acc0[:], 0.0)

            # gold-histogram inputs (consumed from chunk 1 onward, off-path)
            iota_sb = cpool.tile([128, L], f32)
            nc.gpsimd.dma_start(iota_sb[:], iota_d[:])
            goldf_sb = cpool.tile([128, 32], f32)
            nc.gpsimd.dma_start(goldf_sb[:], goldf_d[:])
            sc0_sb = cpool.tile([128, 32], f32)
            nc.gpsimd.dma_start(sc0_sb[:], sc0_d[:])

            # ---- twin 256-step chains, interleaved ----
            cbmap = {}
            w_prev = None     # chain A state (PSUM)
            q_prev = None     # chain B state (PSUM)
            yA = yB = None
            pending = []      # (round_in_chunk, fn): z/feedback ops deferred
                              # into the next chunk, one engine-op per round,
                              # so no queue head-blocks at the chunk seam
            hist_cc = 0
            hist_acc = acc0
            for c in range(NCH2):
                K = CHS[c]
                s0, s1 = SOFF[c], SOFF[c] + K
                if c == 0:
                    rawA, rawB = raw0A, raw0B
                else:
                    rawA = rpool.tile([L, K * BL], f32, tag="rawA")
                    nc.sync.dma_start(rawA[:], s_dram[0:L, s0 * BL:s1 * BL])
                    rawB = rpool.tile([L, K * BL], f32, tag="rawB")
                    nc.sync.dma_start(
                        rawB[:], s_dram[L:2 * L, s0 * BL:s1 * BL])
                seA = epool.tile([L, K, BL], f32, tag="seA")
                nc.scalar.activation(
                    seA[:].rearrange("p a b -> p (a b)"), rawA[:], AF.Exp,
                    bias=negd48[:])
                seB = epool.tile([L, K, BL], f32, tag="seB")
                nc.scalar.activation(
                    seB[:].rearrange("p a b -> p (a b)"), rawB[:], AF.Exp,
                    bias=negd48[:])

                if c in cbmap:
                    cbt = cbmap.pop(c)
                    s0cA = smpool.tile([L, BL], f32, tag="s0cA")
                    nc.vector.tensor_tensor(
                        s0cA[:], seA[:, 0, :], cbt[:, 0:BL], AL.mult)
                    s0cB = smpool.tile([L, BL], f32, tag="s0cB")
                    nc.vector.tensor_tensor(
                        s0cB[:], seB[:, 0, :], cbt[:, BL:2 * BL], AL.mult)
                else:
                    s0cA = s0cB = None

                for k in range(K):
                    while pending and pending[0][0] == k:
                        pending.pop(0)[1]()

                    first = (c == 0 and k == 0)
                    last = (c == NCH2 - 1 and k == K - 1)

                    # chain A, step = s0 + k (ascending t)
                    sA = s0cA[:] if (k == 0 and s0cA is not None) else seA[:, k, :]
                    yA = ypool.tile([L, BL], bf16, tag="yA")
                    if first:
                        nc.vector.tensor_scalar_mul(yA[:], sA, ETcol[:])
                    else:
                        nc.vector.tensor_tensor(yA[:], w_prev[:], sA, AL.mult)
                    w_prev = wpool.tile([L, BL], f32, tag="w")
                    nc.tensor.matmul(
                        w_prev[:], E_st[:], yA[:], start=True, stop=True)

                    # chain B, step t = T-1 - (s0 + k) (host pre-reversed)
                    sB = s0cB[:] if (k == 0 and s0cB is not None) else seB[:, k, :]
                    yB = ypool.tile([L, BL], f32 if last else bf16, tag="yB")
                    if first:
                        nc.vector.tensor_scalar_mul(yB[:], sB, Ecol[:])
                    else:
                        nc.vector.tensor_tensor(yB[:], q_prev[:], sB, AL.mult)
                    if not last:    # r_{T/2} itself never enters a matmul
                        q_prev = qpool.tile([L, BL], f32, tag="q")
                        nc.tensor.matmul(
                            q_prev[:], ET_st[:], yB[:], start=True, stop=True)

                    # gold-histogram pieces folded into the loop's DVE slack
                    if c >= 1 and k % 4 == 2 and hist_cc < 32:
                        cc = hist_cc
                        hist_cc += 1
                        nacc = accpool.tile([128, L], f32, tag="acc")
                        nc.vector.scalar_tensor_tensor(
                            nacc[:], iota_sb[:], goldf_sb[:, cc:cc + 1],
                            hist_acc[:], AL.is_equal, AL.add)
                        hist_acc = nacc
                        if cc == 31:
                            # reduce + sc0 epilogue, inside the loop so it
                            # overlaps the remaining chunks
                            cnt_ps = cntpool.tile([1, L], f32)
                            nc.tensor.matmul(
                                cnt_ps[:], ones128f[:], hist_acc[:],
                                start=True, stop=True)
                            nc.vector.tensor_copy(
                                zbuf[0:1, HC:HC + L], cnt_ps[:])
                            red = smpool.tile([128, 1], f32, tag="red")
                            nc.vector.reduce_sum(red[:], sc0_sb[:], axis=AX.X)
                            sc_ps = zpool.tile([1, 1], f32, tag="z")
                            nc.tensor.matmul(
                                sc_ps[:], red[:], ones128f[:],
                                start=True, stop=True)
                            nc.vector.tensor_copy(
                                zbuf[0:1, ZS:ZS + 1], sc_ps[:])

                # chunk-end magnitude measurement + delayed 1/z feedback,
                # spread one op per round into the next chunk
                if c < NZ:
                    zt = zpool.tile([1, 2 * BL], f32, tag="z")
                    zr = smpool.tile([1, 2 * BL], bf16, tag="zr")
                    cbt = cbpool.tile([L, 2 * BL], f32, tag="cb")

                    def _za(ytile=yA, zt=zt):
                        nc.tensor.matmul(
                            zt[:, 0:BL], ones48b[:], ytile[:],
                            start=True, stop=True)

                    def _zb(ytile=yB, zt=zt):
                        nc.tensor.matmul(
                            zt[:, BL:2 * BL], ones48b[:], ytile[:],
                            start=True, stop=True)

                    def _zc(zt=zt, c=c):
                        nc.vector.tensor_copy(
                            zbuf[0:1, c * 2 * BL:(c + 1) * 2 * BL], zt[:])

                    def _rec(zt=zt, zr=zr):
                        with nc.allow_low_precision(reason="renorm factor"):
                            nc.vector.reciprocal(zr[:], zt[:])

                    def _cb(zr=zr, cbt=cbt):
                        nc.tensor.matmul(
                            cbt[:], ones1x48[:], zr[:], start=True, stop=True)

                    pending = [(1, _za), (2, _zb), (3, _zc),
                               (5, _rec), (6, _cb)]
                    cbmap[c + LAG] = cbt

            # junction dot product: e_END^T w_T = r_{T/2}^T w_{T/2}
            dprod = smpool.tile([L, BL], f32, tag="dprod")
            nc.vector.tensor_tensor(dprod[:], w_prev[:], yB[:], AL.mult)
            d_ps = zpool.tile([1, BL], f32, tag="z")
            nc.tensor.matmul(d_ps[:], ones48f[:], dprod[:], start=True, stop=True)
            nc.vector.tensor_copy(zbuf[0:1, ZD:ZD + BL], d_ps[:])
            nc.sync.dma_start(out_d[:], zbuf[:])

    nc.compile()
    return nc


def _get_nc():
    if "nc" not in _NC_CACHE:
        _NC_CACHE["nc"] = build_nc()
    return _NC_CACHE["nc"]


def make_in_maps(scores, gold_target, transitions):
    scores = np.asarray(scores, dtype=np.float32)
    gold = np.asarray(gold_target)
    trans = np.ascontiguousarray(np.asarray(transitions, dtype=np.float32))
    transT = np.ascontiguousarray(trans.T)
    iota = np.ascontiguousarray(
        np.broadcast_to(np.arange(L, dtype=np.float32)[None, :], (128, L)))

    in_maps = []
    for c in range(NCORES):
        sc = scores[c * BL:(c + 1) * BL]                     # (BL, T, L)
        sA = np.ascontiguousarray(sc.transpose(2, 1, 0))     # (L, T, BL)
        s_tr = np.empty((2 * L, H * BL), dtype=np.float32)
        s_tr[0:L] = sA[:, :H, :].reshape(L, H * BL)
        s_tr[L:2 * L] = sA[:, ::-1, :][:, :H, :].reshape(L, H * BL)
        goldf = np.ascontiguousarray(
            gold[c * BL:(c + 1) * BL].astype(np.float32).reshape(128, 32))
        sc0 = np.ascontiguousarray(sc[:, :, 0].astype(np.float32).reshape(128, 32))
        in_maps.append({
            "s_tr": s_tr, "trans": trans, "transT": transT,
            "goldf": goldf, "sc0": sc0, "iotaf": iota,
        })
    return in_maps


def combine_outputs(results, transitions):
    trans = np.asarray(transitions, dtype=np.float64)
    forscore = 0.0
    tg_energy = 0.0
    ZD = NZ * 2 * BL
    ZS = ZD + BL
    HC = ZS + 1
    for c in range(NCORES):
        out = np.asarray(results[c]["out_all"], dtype=np.float64)[0]
        zv = out[:ZD].reshape(NZ, 2, BL)
        dv = out[ZD:ZD + BL]
        sc0s = out[ZS]
        cnt = out[HC:HC + L]
        fs_end = (np.log(dv) + DELTA * T + np.log(zv).sum(axis=(0, 1)))
        forscore += fs_end.sum()
        tg_energy += (cnt * trans[0, :]).sum() + sc0s + BL * trans[0, START]
    return np.float32((forscore - tg_energy) / B)


def kernel(scores, gold_target, mask, transitions):
    from concourse.bass_utils import run_bass_kernel_spmd

    nc = _get_nc()
    in_maps = make_in_maps(scores, gold_target, transitions)
    res = run_bass_kernel_spmd(nc, in_maps, list(range(NCORES)))
    return combine_outputs(res.results, transitions)
